# revision 46
# baseline (speedup 1.0000x reference)
"""CAB (channel-attention block) Trainium2 kernel, bf16 datapath.

Sharding: 8 cores = 4 batches x 2 H-halves. Each core computes its
[64, 128, 256] slice of the output. The q.kT contraction and the L2-norm
sums are AllReduced across the 2 cores sharing a batch (sequence-parallel).

Math folds (all exact):
  - L2 normalize folds into S: attn_logits = S * temp / (||q|| ||k||^T),
    with S = q @ k^T computed on raw (unnormalized) q, k.
  - proj o (attn @ v) o dwconv_v o pwconv_v folds into a single 3x3 dense
    conv on input2 with data-dependent matrices
    G[dp] = (P @ A_blockdiag) @ (diag(wdv[:,dp]) @ Wv).

PE packing:
  - pass 1: q-conv (x1) and k-conv (x2) run as ONE matmul per tap with a
    block-diagonal [128,128] lhsT and rhs = [x1; x2] stacked on partitions
    (K=128, M=128) -> 9 PE cycles/col for both convs.
  - pass 2: rhs = [x2 @ off; x2 @ off+R] (row-shifted copies on the two
    partition halves); each [128,128] lhsT computes TWO output rows at
    once -> 6 matmuls per row-pair (3 PE cycles/col).
"""
import sys

sys.path.insert(0, "/opt/trn_rl_repo")

import numpy as np
import ml_dtypes

import concourse.bacc as bacc
import concourse.bass as bass
import concourse.tile as tile
from concourse import mybir
from concourse.bass_utils import run_bass_kernel_spmd

F32 = mybir.dt.float32
BF16 = mybir.dt.bfloat16
NP_BF16 = ml_dtypes.bfloat16

B, C, H, W = 4, 64, 256, 256
HEADS = 8
HD = C // HEADS
EPS = 1e-12

HH = H // 2            # rows per core
R = W + 2              # padded row length
NR = HH + 4            # buffer rows: zero, halo, 128 data, halo, zero
NBUF = NR * R          # per-core padded input length (flattened)
P0 = 2 * R             # first output position (data row 0, col -1(pad))
NOUT = HH * R          # output span incl. per-row col pads

TAPS = [(dy, dx) for dy in (-1, 0, 1) for dx in (-1, 0, 1)]
# rhs offset of tap within a window that starts 259 cols before the chunk
TAP_OFF = [259 + dy * R + dx for dy, dx in TAPS]

SPAN1 = 8192           # pass-1 window span (multiple of 128)
CHUNK = 512            # pass-1 psum chunk (one bank)
PAIRS2 = 8             # pass-2 row-pairs per window

_CACHE = {}


def _pad_positions(start, length):
    """Contiguous runs of pad columns (global col % R in {0, R-1}) within
    [start, start+length), as (offset_rel, run_len) with runs <= 2."""
    runs = []
    end = start + length
    p = (start // R) * R - 1
    while p < end:
        for q in (p, p + 1):  # (row r col 257, row r+1 col 0) adjacent pair
            lo = max(q, start)
            hi = min(q + 1, end)
            if lo < hi:
                if runs and runs[-1][0] + runs[-1][1] == lo - start:
                    runs[-1] = (runs[-1][0], runs[-1][1] + (hi - lo))
                else:
                    runs.append((lo - start, hi - lo))
        p += R
    return runs


def build_module():
    nc = bacc.Bacc("TRN2", target_bir_lowering=False, debug=False, num_devices=8)

    x1 = nc.declare_dram_parameter("x1", [C, NBUF], BF16, isOutput=False)
    x2 = nc.declare_dram_parameter("x2", [C, NBUF], BF16, isOutput=False)
    lqk9 = nc.declare_dram_parameter("lqk9", [128, 9 * 128], BF16, isOutput=False)
    lqk9s = nc.declare_dram_parameter("lqk9s", [128, 9 * 64], BF16, isOutput=False)
    wv9 = nc.declare_dram_parameter("wv9", [C, 9 * C], F32, isOutput=False)
    wv6 = nc.declare_dram_parameter("wv6", [128, 6 * 128], BF16,
                                    isOutput=False)
    pT = nc.declare_dram_parameter("pT", [C, C], F32, isOutput=False)
    temp64 = nc.declare_dram_parameter("temp64", [C, 1], F32, isOutput=False)
    ident = nc.declare_dram_parameter("ident", [128, 128], F32, isOutput=False)
    identb = nc.declare_dram_parameter("identb", [128, 128], BF16, isOutput=False)
    mask64 = nc.declare_dram_parameter("mask64", [C, C], F32, isOutput=False)
    y = nc.declare_dram_parameter("y", [C, HH, W], BF16, isOutput=True)

    with tile.TileContext(nc) as tc:
        _body(tc, nc, x1, x2, lqk9, lqk9s, wv9, wv6, pT, temp64, ident,
              identb, mask64, y)
    nc.compile()
    return nc


def _body(tc, nc, x1, x2, lqk9, lqk9s, wv9, wv6, pT, temp64, ident,
          identb, mask64, y):
    mm = nc.tensor.matmul
    f = F32

    wpool = tc.alloc_tile_pool(name="weights", bufs=1)
    dram = tc.alloc_tile_pool(name="dram", bufs=1, space="DRAM")
    accp = tc.alloc_tile_pool(name="ps_acc", bufs=1, space=bass.MemorySpace.PSUM)
    persist = tc.alloc_tile_pool(name="persist", bufs=1)

    # only the pass-1 weights load up-front, on the scalar queue so they
    # run in parallel with the first x-window DMA (gpsimd queue)
    w_lqk9 = wpool.tile([128, 9 * 128], BF16)
    nc.scalar.dma_start(w_lqk9[:], lqk9[:])
    w_lqk9s = wpool.tile([128, 9 * 64], BF16)
    nc.scalar.dma_start(w_lqk9s[:], lqk9s[:])
    w_idb = wpool.tile([128, 128], BF16)
    nc.scalar.dma_start(w_idb[:], identb[:])
    w_wv6 = wpool.tile([128, 6 * 128], BF16)
    nc.scalar.dma_start(w_wv6[:], wv6[:])
    w_pT = wpool.tile([C, C], f)
    w_temp = wpool.tile([C, 1], f)
    w_id = wpool.tile([128, 128], f)
    w_mask = wpool.tile([C, C], f)

    acc_ps = accp.tile([C, C], f)          # S accumulator (q.kT)
    qk2e = persist.tile([128, 2], f)       # sum-of-squares, col0: even
    nc.vector.memset(qk2e[:], 0.0)         # 512-blocks ([q;k]), col1: odd

    # pass-2 window prefetch machinery (tiles loaded before the collective
    # so the DMA overlaps the AllReduce latency)
    n_pairs = HH // 2
    n_win2 = (n_pairs + PAIRS2 - 1) // PAIRS2
    xw2p = tc.alloc_tile_pool(name="xw2", bufs=n_win2)
    xw2_tiles = {}

    W2COLS = (2 * PAIRS2 + 1) * R

    def load_win2(wi):
        # partitions 0:64 <- x2 rows (rr-1)..(rr+15); 64:128 <- one row down
        rr = 2 * wi * PAIRS2
        s0 = (rr + 1) * R
        # split across the scalar and sync DMA queues: pass-1 loads occupy
        # gpsimd's queue, and a single queue only sustains ~50 GB/s
        xw = xw2p.tile([128, W2COLS], BF16)
        nc.scalar.dma_start(xw[0:C, :], x2[:, s0:s0 + W2COLS])
        nc.scalar.dma_start(xw[C:128, :], x2[:, s0 + R:s0 + R + W2COLS])
        xw2_tiles[wi] = xw

    # ---------------- pass 1: q,k conv -> transpose -> S, norms ----------
    n_blk_total = NOUT // 128
    windows = []
    ws_ = 0
    ramp = [256, 1024]         # small first windows -> PE starts sooner
    while ws_ < NOUT:
        span = ramp[len(windows)] if len(windows) < len(ramp) else SPAN1
        windows.append((ws_, min(span, NOUT - ws_)))
        ws_ += windows[-1][1]

    with (
        tc.tile_pool(name="xw1", bufs=2) as xw1p,
        tc.tile_pool(name="qkwin", bufs=2) as qkwp,
        tc.tile_pool(name="trsb", bufs=3) as trsbp,
        tc.tile_pool(name="scratch", bufs=1) as scrp,
        tc.tile_pool(name="acct", bufs=2) as acctp,
        tc.tile_pool(name="ps_cva", bufs=2, space=bass.MemorySpace.PSUM) as pcva,
        tc.tile_pool(name="ps_cvb", bufs=2, space=bass.MemorySpace.PSUM) as pcvb,
        tc.tile_pool(name="ps_cvt", bufs=1, space=bass.MemorySpace.PSUM) as pcvt,
        tc.tile_pool(name="ps_tr", bufs=2, space=bass.MemorySpace.PSUM) as ptr,
    ):
        scratch = scrp.tile([128, SPAN1], BF16)
        blk_idx = 0
        prev_groups = []   # (qkwin, j0, wj) transpose groups of prev window
        tq = []            # (trsb, nb) copied groups awaiting S-matmuls

        def emit_T():
            # 4 transposes batched into one psum bank -> one copy
            qkwin, j0, wj = prev_groups.pop(0)
            nb = wj // 128
            trps = ptr.tile([128, 512], BF16, tag="trps")
            for j in range(nb):
                nc.tensor.transpose(trps[:, j * 128:(j + 1) * 128],
                                    qkwin[:, j0 + j * 128:j0 + (j + 1) * 128],
                                    w_idb[:])
            trsb = trsbp.tile([128, 512], BF16)
            nc.scalar.copy(trsb[:, 0:wj], trps[:, 0:wj])
            tq.append((trsb, nb, (j0 // 512) % 2))

        def emit_S():
            nonlocal blk_idx
            trsb, nb, par = tq.pop(0)
            for j in range(nb):
                qs = trsb[:, j * 128 + par * C:j * 128 + par * C + C]
                ks = trsb[:, j * 128 + (1 - par) * C:
                          j * 128 + (1 - par) * C + C]
                mm(acc_ps[:], qs, ks,
                   start=(blk_idx == 0), stop=(blk_idx == n_blk_total - 1))
                blk_idx += 1

        for widx, (wstart, width) in enumerate(windows):
            p_start = P0 + wstart
            ws = p_start - 259
            wwidth = width + 518
            xw = xw1p.tile([128, SPAN1 + 518], BF16)
            nc.gpsimd.dma_start(xw[0:C, 0:wwidth], x1[:, ws:ws + wwidth])
            nc.sync.dma_start(xw[C:128, 0:wwidth], x2[:, ws:ws + wwidth])

            qkwin = qkwp.tile([128, SPAN1], BF16)
            # conv super-chunks of 1024 cols: 4 concurrent 64x64 matmuls
            # (tile_position quadrants) fill the whole PE grid -> 4.5
            # cycles/col for both convs. q lands on psum partitions 0:64
            # for the even 512-block and 64:128 for the odd one (k vice
            # versa) so every psum->sbuf copy stays lane-aligned; qkwin's
            # odd blocks hold [k; q] and the S-matmuls swap operands there.
            lc = 0
            while lc < width:
                if lc + 1024 <= width:
                    qa = pcva.tile([128, CHUNK], f, tag="qa")
                    qb = pcvb.tile([128, CHUNK], f, tag="qb")
                    for t in range(9):
                        o = lc + TAP_OFF[t]
                        wq = w_lqk9s[0:C, t * C:(t + 1) * C]
                        wk = w_lqk9s[C:128, t * C:(t + 1) * C]
                        st, sp = (t == 0), (t == 8)
                        mm(qa[0:C, :], wq, xw[0:C, o:o + 512],
                           start=st, stop=sp, tile_position=(0, 0))
                        mm(qa[C:128, :], wq, xw[0:C, o + 512:o + 1024],
                           start=st, stop=sp, tile_position=(0, 64))
                        mm(qb[C:128, :], wk, xw[C:128, o:o + 512],
                           start=st, stop=sp, tile_position=(64, 64))
                        mm(qb[0:C, :], wk, xw[C:128, o + 512:o + 1024],
                           start=st, stop=sp, tile_position=(64, 0))
                    nc.vector.tensor_copy(qkwin[0:C, lc:lc + 512], qa[0:C, :])
                    nc.vector.tensor_copy(qkwin[C:128, lc:lc + 512],
                                          qb[C:128, :])
                    nc.vector.tensor_copy(qkwin[0:C, lc + 512:lc + 1024],
                                          qb[0:C, :])
                    nc.vector.tensor_copy(qkwin[C:128, lc + 512:lc + 1024],
                                          qa[C:128, :])
                    step = 1024
                else:
                    L = min(CHUNK, width - lc)
                    qk_ps = pcvt.tile([128, CHUNK], f, tag="qkps")
                    for t in range(9):
                        o = lc + TAP_OFF[t]
                        mm(qk_ps[:, 0:L], w_lqk9[:, t * 128:(t + 1) * 128],
                           xw[:, o:o + L], start=(t == 0), stop=(t == 8))
                    nc.vector.tensor_copy(qkwin[:, lc:lc + L], qk_ps[:, 0:L])
                    step = L
                lc += step
                for _ in range(step // 512):
                    if prev_groups:
                        emit_T()
                    if len(tq) >= 2:
                        emit_S()

            # zero the per-row pad columns so they don't pollute S / norms
            for off, ln in _pad_positions(p_start, width):
                nc.gpsimd.memset(qkwin[:, off:off + ln], 0.0)

            # norms: sum of squares per 512-block parity
            acc_tmp = acctp.tile([128, 2], f)
            nsup = width // 1024
            if nsup == 0:
                nc.scalar.activation(
                    scratch[:, 0:width], qkwin[:, 0:width],
                    mybir.ActivationFunctionType.Square,
                    accum_out=acc_tmp[:, 0:1])
                nc.vector.tensor_add(qk2e[:, 0:1], qk2e[:, 0:1],
                                     acc_tmp[:, 0:1])
            else:
                qdim = list(qkwin.ap)[0]
                sdim = list(scratch.ap)[0]
                for par in (0, 1):
                    ina = bass.AP(qkwin.tensor, qkwin.offset + par * 512,
                                  [qdim, [1024, nsup], [1, 512]])
                    outa = bass.AP(scratch.tensor, scratch.offset + par * 512,
                                   [sdim, [1024, nsup], [1, 512]])
                    nc.scalar.activation(
                        outa, ina, mybir.ActivationFunctionType.Square,
                        accum_out=acc_tmp[:, par:par + 1])
                nc.vector.tensor_add(qk2e[:], qk2e[:], acc_tmp[:])

            while prev_groups:   # ragged window: drain leftovers
                emit_T()
                if len(tq) >= 2:
                    emit_S()
            prev_groups.extend(
                (qkwin, j0, min(512, width - j0))
                for j0 in range(0, width, 512))

            # stream one pass-2 window load per pass-1 window; by the
            # collective, all pass-2 inputs are SBUF-resident
            if widx >= 1 and (widx - 1) < n_win2:
                load_win2(widx - 1)

        while prev_groups or tq:
            if prev_groups:
                emit_T()
            if tq:
                emit_S()

    # any pass-2 windows not yet streamed during pass 1, plus mid weights
    for wi in range(len(windows) - 1, n_win2):
        load_win2(wi)
    nc.scalar.dma_start(w_pT[:], pT[:])
    nc.scalar.dma_start(w_temp[:], temp64[:])
    nc.scalar.dma_start(w_id[:], ident[:])
    nc.scalar.dma_start(w_mask[:], mask64[:])

    # ---------------- collective: S and norms over the batch pair --------
    cc_sb = persist.tile([128, C + 2], f)
    nc.vector.memset(cc_sb[:], 0.0)
    nc.scalar.copy(cc_sb[0:C, 0:C], acc_ps[:])
    nc.vector.tensor_copy(cc_sb[:, C:C + 2], qk2e[:])
    cc_in = dram.tile([128, C + 2], f)
    cc_out = dram.tile([128, C + 2], f, tag="cc_out")
    nc.sync.dma_start(cc_in[:], cc_sb[:])
    nc.gpsimd.collective_compute(
        "AllReduce", mybir.AluOpType.add,
        replica_groups=[[0, 1], [2, 3], [4, 5], [6, 7]],
        ins=[cc_in.opt()], outs=[cc_out.opt()],
    )
    # ---- v = dwconv(pwconv(x2)) while the AllReduce is in flight ----
    # (attn-independent: out = (P@A) @ v, so only the tiny pointwise
    # M-sweep must wait for the collective; this fills the AR-skew gap)
    vbuf = persist.tile([128, n_pairs, W], BF16, tag="vbuf")
    with tc.tile_pool(name="ps_v", bufs=4, space=bass.MemorySpace.PSUM) as pv:
        for wi in range(n_win2):
            base_pair = wi * PAIRS2
            np_w = min(PAIRS2, n_pairs - base_pair)
            xw = xw2_tiles.pop(wi)
            pdim = list(xw.ap)[0]

            def rhs2(col, xw=xw, pdim=pdim):
                return bass.AP(xw.tensor, xw.offset + col,
                               [pdim, [2 * R, 2], [1, W]])

            for p in range(0, np_w, 2):
                ps3 = pv.tile([128, 2, W], f, tag="vps")
                for i, dx in enumerate((-1, 0, 1)):
                    va = w_wv6[:, (2 * i) * 128:(2 * i + 1) * 128]
                    vb = w_wv6[:, (2 * i + 1) * 128:(2 * i + 2) * 128]
                    mm(ps3[:], va, rhs2((2 * p) * R + dx + 1),
                       start=(i == 0), stop=False)
                    mm(ps3[:], vb, rhs2((2 * p + 2) * R + dx + 1),
                       start=False, stop=(i == 2))
                gp = base_pair + p
                nc.vector.tensor_copy(vbuf[:, gp:gp + 2, :], ps3[:])

    sqk = persist.tile([128, C + 2], f, tag="sqk")
    nc.sync.dma_start(sqk[:], cc_out[:])

    # ------------- tiny mid-section: softmax, M = P@Ablk block-diag ------
    mdiag = persist.tile([128, 128], BF16, tag="mdiag")
    with (
        tc.tile_pool(name="mid", bufs=1) as midp,
        tc.tile_pool(name="ps_mid", bufs=1, space=bass.MemorySpace.PSUM) as pmid,
    ):
        # untangle parity-split sums: ||q_c||^2 = even[c] + odd[64+c]
        # (k vice versa) -> v = even + swap64(odd), swap via transposes
        ro_ps = pmid.tile([1, 128], f, tag="ro")
        nc.tensor.transpose(ro_ps[:], sqk[:, C + 1:C + 2], w_id[:])
        ro = midp.tile([1, 128], f, tag="ro_sb")
        nc.scalar.copy(ro[:], ro_ps[:])
        rsw = midp.tile([1, 128], f, tag="rsw")
        nc.vector.tensor_copy(rsw[:, 0:C], ro[:, C:128])
        nc.vector.tensor_copy(rsw[:, C:128], ro[:, 0:C])
        rvs_ps = pmid.tile([128, 1], f, tag="rvs")
        nc.tensor.transpose(rvs_ps[:], rsw[:], w_id[0:1, 0:1])
        qk2v = midp.tile([128, 1], f, tag="qk2v")
        nc.vector.tensor_add(qk2v[:], sqk[:, C:C + 1], rvs_ps[:])
        nrm = midp.tile([128, 1], f, tag="nrm")       # sqrt of sums
        nc.scalar.sqrt(nrm[:], qk2v[:])
        nc.vector.tensor_scalar_max(nrm[:], nrm[:], EPS)
        rn = midp.tile([128, 1], f, tag="rn")         # 1/||.||
        nc.vector.reciprocal(rn[:], nrm[:])
        rs = midp.tile([C, 1], f, tag="rs")           # temp/||q|| per row c
        nc.vector.tensor_mul(rs[:], rn[0:C, :], w_temp[:])

        # broadcast 1/||k|| along free dim: transpose then rank-1 outer
        nkT_ps = pmid.tile([1, C], f, tag="nkT")
        nc.tensor.transpose(nkT_ps[:], rn[C:128, :], w_id[C:128, C:128])
        nkT = midp.tile([1, C], f, tag="nkT_sb")
        nc.scalar.copy(nkT[:], nkT_ps[:])
        ones1 = midp.tile([1, C], f, tag="ones1")
        nc.vector.memset(ones1[:], 1.0)
        nkb_ps = pmid.tile([C, C], f, tag="nkb")
        mm(nkb_ps[:], ones1[:], nkT[:])
        # logits = S * rs(row) * (1/||k||)(col)
        sp = midp.tile([C, C], f, tag="sp")
        nc.vector.tensor_scalar(sp[:], sqk[0:C, 0:C], rs[:], None,
                                op0=mybir.AluOpType.mult)
        nc.vector.tensor_mul(sp[:], sp[:], nkb_ps[:])

        # blockwise softmax via additive off-block mask (-1e30):
        # off-block entries exp to exactly 0, so the result IS Ablk.
        nc.vector.tensor_add(sp[:], sp[:], w_mask[:])
        negm = midp.tile([C, 1], f, tag="negm")
        nc.vector.tensor_reduce(negm[:], sp[:], axis=mybir.AxisListType.X,
                                op=mybir.AluOpType.max, negate=True)
        den = midp.tile([C, 1], f, tag="den")
        ex = midp.tile([C, C], f, tag="ex")
        nc.scalar.activation(ex[:], sp[:], mybir.ActivationFunctionType.Exp,
                             bias=negm[:], scale=1.0, accum_out=den[:])
        rden = midp.tile([C, 1], f, tag="rden")
        nc.vector.reciprocal(rden[:], den[:])
        ablk = midp.tile([C, C], f, tag="ablk")
        nc.vector.tensor_scalar(ablk[:], ex[:], rden[:], None,
                                op0=mybir.AluOpType.mult)

        # M = P @ Ablk as block-diagonal bf16 lhsT (both halves)
        mt_ps = pmid.tile([128, C], f, tag="mt")
        mm(mt_ps[0:C, :], ablk[:], w_pT[:], tile_position=(0, 0))
        mm(mt_ps[C:128, :], ablk[:], w_pT[:], tile_position=(0, 64))
        nc.vector.memset(mdiag[:], 0.0)
        nc.scalar.copy(mdiag[0:C, 0:C], mt_ps[0:C, :])
        nc.scalar.copy(mdiag[C:128, C:128], mt_ps[C:128, :])

    accp.release()

    # ---------------- pass 2: out = M @ v (pointwise), write y -----------
    with (
        tc.tile_pool(name="osb", bufs=4) as osbp,
        tc.tile_pool(name="ps_p2", bufs=4, space=bass.MemorySpace.PSUM) as pp2,
    ):
        for p in range(0, n_pairs, 2):
            ps3 = pp2.tile([128, 2, W], f, tag="o2")
            mm(ps3[:], mdiag[:], vbuf[:, p:p + 2, :], start=True, stop=True)
            obs3 = osbp.tile([128, 2, W], BF16)
            nc.vector.tensor_copy(obs3[:], ps3[:])
            row = 2 * p
            nc.sync.dma_start(y[:, row:row + 4:2, :], obs3[0:C])
            nc.sync.dma_start(y[:, row + 1:row + 4:2, :], obs3[C:128])

    xw2p.release()
    for p in (persist, dram, wpool):
        p.release()


# ======================= host side =========================================

def _prep_consts(q_w, q_dw_w, kv_w, kv_dw_w, proj_w, temperature):
    q_w = np.asarray(q_w, np.float32)[:, :, 0, 0]          # [o, i]
    kv_w = np.asarray(kv_w, np.float32)[:, :, 0, 0]        # [2C, i]
    q_dw = np.asarray(q_dw_w, np.float32)[:, 0]            # [C, 3, 3]
    kv_dw = np.asarray(kv_dw_w, np.float32)[:, 0]          # [2C, 3, 3]
    proj = np.asarray(proj_w, np.float32)[:, :, 0, 0]      # [o, c]
    temp = np.asarray(temperature, np.float32).reshape(HEADS)

    lqk9 = np.zeros((128, 9 * 128), np.float32)
    lqk9s = np.zeros((128, 9 * 64), np.float32)
    wv9 = np.zeros((C, 9 * C), np.float32)
    for t, (dy, dx) in enumerate(TAPS):
        w9q = q_dw[:, dy + 1, dx + 1][:, None] * q_w       # [o, i]
        w9k = kv_dw[0:C, dy + 1, dx + 1][:, None] * kv_w[0:C]
        lqk9[0:C, t * 128:t * 128 + C] = w9q.T
        lqk9[C:128, t * 128 + C:(t + 1) * 128] = w9k.T
        lqk9s[0:C, t * C:(t + 1) * C] = w9q.T
        lqk9s[C:128, t * C:(t + 1) * C] = w9k.T
        # wv9[dp][d, i] = wdv[d, dp] * Wv[d, i]
        wv9[:, t * C:(t + 1) * C] = (
            kv_dw[C:2 * C, dy + 1, dx + 1][:, None] * kv_w[C:2 * C]
        )
    wv6 = np.zeros((128, 6 * 128), np.float32)

    def wvt(dy, dx):
        return (kv_dw[C:2 * C, dy + 1, dx + 1][:, None] * kv_w[C:2 * C]).T

    for ix, dx in enumerate((-1, 0, 1)):
        A = np.zeros((128, 128), np.float32)
        B = np.zeros((128, 128), np.float32)
        A[0:C, 0:C] = wvt(-1, dx)
        A[C:128, 0:C] = wvt(0, dx)
        A[C:128, C:128] = wvt(-1, dx)
        B[0:C, 0:C] = wvt(1, dx)
        B[0:C, C:128] = wvt(0, dx)
        B[C:128, C:128] = wvt(1, dx)
        wv6[:, (2 * ix) * 128:(2 * ix + 1) * 128] = A
        wv6[:, (2 * ix + 1) * 128:(2 * ix + 2) * 128] = B
    pTm = proj.T.copy()                                    # [c, o]
    temp64 = np.repeat(temp, HD).reshape(C, 1).astype(np.float32)
    ident = np.eye(128, dtype=np.float32)
    identb = np.eye(128, dtype=NP_BF16)
    mask = np.full((C, C), -1e30, np.float32)
    for h in range(HEADS):
        mask[h * HD:(h + 1) * HD, h * HD:(h + 1) * HD] = 0.0
    return (lqk9.astype(NP_BF16), lqk9s.astype(NP_BF16), wv9,
            wv6.astype(NP_BF16), pTm, temp64, ident, identb, mask)


def _prep_slices(img_bf):
    """[C, H, W] bf16 -> padded flat [C, NBUF] per half; returns (top, bot)."""
    out = []
    for h in range(2):
        buf = np.zeros((C, NR, R), NP_BF16)
        r0 = h * HH
        lo, hi = r0 - 1, r0 + HH + 1
        vlo, vhi = max(lo, 0), min(hi, H)
        buf[:, 1 + (vlo - lo):1 + (vlo - lo) + (vhi - vlo), 1:W + 1] = \
            img_bf[:, vlo:vhi, :]
        out.append(np.ascontiguousarray(buf.reshape(C, NBUF)))
    return out


def _build_in_maps(input1, input2, q_w, q_dw_w, kv_w, kv_dw_w, proj_w,
                   temperature):
    lqk9, lqk9s, wv9, wv6, pTm, temp64, ident, identb, mask = _prep_consts(
        q_w, q_dw_w, kv_w, kv_dw_w, proj_w, temperature)
    in1_bf = np.asarray(input1, np.float32).astype(NP_BF16)
    in2_bf = np.asarray(input2, np.float32).astype(NP_BF16)
    in_maps = []
    for core in range(8):
        b, h = core // 2, core % 2
        x1t = _prep_slices(in1_bf[b])[h]
        x2t = _prep_slices(in2_bf[b])[h]
        in_maps.append({
            "x1": x1t, "x2": x2t, "lqk9": lqk9, "lqk9s": lqk9s,
            "wv9": wv9, "wv6": wv6, "pT": pTm,
            "temp64": temp64, "ident": ident, "identb": identb,
            "mask64": mask,
        })
    return in_maps


def kernel(input1, input2, q_w, q_dw_w, kv_w, kv_dw_w, proj_w, temperature):
    if "nc" not in _CACHE:
        _CACHE["nc"] = build_module()
    nc = _CACHE["nc"]

    in_maps = _build_in_maps(input1, input2, q_w, q_dw_w, kv_w, kv_dw_w,
                             proj_w, temperature)
    results = _get_runner(nc)(in_maps)
    out = np.empty((B, C, H, W), np.float32)
    for core in range(8):
        b, h = core // 2, core % 2
        out[b, :, h * HH:(h + 1) * HH, :] = \
            results[core]["y"].astype(np.float32)
    return out


def _get_runner(nc, n_cores=8):
    """Like bass2jax.run_bass_via_pjrt, but the jitted shard_map executable is
    built once and reused across calls (avoids per-call retracing)."""
    if "runner" in _CACHE:
        return _CACHE["runner"]
    import jax
    from jax.sharding import Mesh, PartitionSpec
    from jax.experimental.shard_map import shard_map
    from concourse import bass2jax as b2j
    from concourse import mybir as _mb

    b2j.install_neuronx_cc_hook()
    partition_name = nc.partition_id_tensor.name if nc.partition_id_tensor else None
    in_names, out_names, out_avals, zero_shapes = [], [], [], []
    for alloc in nc.m.functions[0].allocations:
        if not isinstance(alloc, _mb.MemoryLocationSet):
            continue
        name = alloc.memorylocations[0].name
        if alloc.kind == "ExternalInput":
            if name != partition_name:
                in_names.append(name)
        elif alloc.kind == "ExternalOutput":
            out_names.append(name)
            shape = tuple(alloc.tensor_shape)
            dtype = _mb.dt.np(alloc.dtype)
            out_avals.append(jax.core.ShapedArray(shape, dtype))
            zero_shapes.append((shape, dtype))
    n_params = len(in_names)
    n_outs = len(out_avals)
    all_in_names = list(in_names) + list(out_names)
    if partition_name is not None:
        all_in_names.append(partition_name)
    donate = tuple(range(n_params, n_params + n_outs))

    def _pjrt_body(*args):
        operands = list(args)
        if partition_name is not None:
            operands.append(b2j.partition_id_tensor())
        return tuple(b2j._bass_exec_p.bind(
            *operands, out_avals=tuple(out_avals), in_names=tuple(all_in_names),
            out_names=tuple(out_names), lowering_input_output_aliases=(),
            sim_require_finite=True, sim_require_nnan=True, nc=nc))

    devices = jax.devices()[:n_cores]
    mesh = Mesh(np.asarray(devices), ("core",))
    sharded = jax.jit(
        shard_map(_pjrt_body, mesh=mesh,
                  in_specs=(PartitionSpec("core"),) * (n_params + n_outs),
                  out_specs=(PartitionSpec("core"),) * n_outs, check_rep=False),
        donate_argnums=donate, keep_unused=True)

    def run(in_maps):
        concat_in = [
            np.concatenate([np.asarray(in_maps[c][nm]) for c in range(n_cores)], 0)
            for nm in in_names
        ]
        concat_zeros = [np.zeros((n_cores * s[0], *s[1:]), d)
                        for s, d in zero_shapes]
        out_arrs = sharded(*concat_in, *concat_zeros)
        return [
            {nm: np.asarray(out_arrs[i]).reshape(n_cores, *out_avals[i].shape)[c]
             for i, nm in enumerate(out_names)}
            for c in range(n_cores)
        ]

    _CACHE["runner"] = run
    return run
